# revision 23
# baseline (speedup 1.0000x reference)
"""Trainium2 Bass kernel for nn_Decoder (embedding + Luong attention + LSTM + FC).

Sharding (8 NeuronCores):
  - Attention + LSTM replicated on every core (the recurrence is latency-bound,
    not throughput-bound, so replication costs no wall-clock and avoids
    per-step collectives).
  - The dominant cost, logits = hs @ Wfc ([2048,512] @ [512,32000], 262MB fp32
    output), is tensor-parallel over the vocab axis: core j computes columns
    [j*4000, (j+1)*4000).

Layout: token-major, t-major ordering (token = t*B + b) so each LSTM timestep
is a contiguous 32-column slice.  The LSTM recurrence runs in transposed
orientation zT[2048, 32] = Wx-part + Wh.T @ hT with Wh stationary (bf16) and
the per-step xmm slice injected into PSUM via an identity matmul.  The xmm =
lstm_in @ Wx product is precomputed for all timesteps (gate-permuted so each
PSUM bank reads one contiguous block) and spilled to DRAM, streamed back one
timestep per step.  Logits matmuls are interleaved into the recurrence's gate
latency gaps, two vocab chunks per step.  Weights arrive pre-cast to bf16
from the host; fp32 is kept for the cell state, biases and all outputs.
"""

import sys

if "/opt/trn_rl_repo" not in sys.path:
    sys.path.insert(0, "/opt/trn_rl_repo")

from contextlib import ExitStack

import ml_dtypes
import numpy as np

import concourse.bass as bass
import concourse.tile as tile
from concourse import bacc, masks, mybir
from concourse.bass import IndirectOffsetOnAxis
from concourse.bass_utils import run_bass_kernel_spmd

F32 = mybir.dt.float32
BF16 = mybir.dt.bfloat16
I32 = mybir.dt.int32
AF = mybir.ActivationFunctionType
ALU = mybir.AluOpType
AX = mybir.AxisListType

V, EMB, DU = 32000, 256, 512
B, T, S = 32, 64, 64          # batch, Tout, Tin
TOK = B * T                   # 2048 tokens
NT = TOK // 128               # 16 token tiles
NCORES = 8
VSH = V // NCORES             # 4000 vocab cols per core
G4 = 4 * DU                   # 2048 gate dims
NM = G4 // 128                # 16 gate-dim chunks
NK = DU // 128                # 4 contraction chunks over DU
NE = EMB // 128               # 2 contraction chunks over EMB
NX = (DU + EMB) // 128        # 6 contraction chunks over DU+EMB
VCH = 500                     # vocab chunk per psum bank
NV = VSH // VCH               # 8 vocab chunks


def build_nc(interleave_logits=True):
    nc = bacc.Bacc()

    inp = nc.dram_tensor("inputs", [B, T], I32, kind="ExternalInput")
    enc = nc.dram_tensor("enc", [B * S, DU], BF16, kind="ExternalInput")
    h0 = nc.dram_tensor("h0", [B, DU], F32, kind="ExternalInput")
    c0 = nc.dram_tensor("c0", [B, DU], F32, kind="ExternalInput")
    emb = nc.dram_tensor("emb", [V, EMB], BF16, kind="ExternalInput")
    wq = nc.dram_tensor("wq", [EMB, DU], BF16, kind="ExternalInput")
    bq = nc.dram_tensor("bq", [DU, 1], F32, kind="ExternalInput")
    wx = nc.dram_tensor("wx", [DU + EMB, G4], BF16, kind="ExternalInput")
    wh = nc.dram_tensor("wh", [DU, G4], BF16, kind="ExternalInput")
    bl = nc.dram_tensor("bl", [G4, 1], F32, kind="ExternalInput")
    wfc = nc.dram_tensor("wfc", [DU, VSH], BF16, kind="ExternalInput")
    bfc = nc.dram_tensor("bfc", [1, VSH], F32, kind="ExternalInput")

    logits_o = nc.dram_tensor("logits", [TOK, VSH], F32, kind="ExternalOutput")
    h_o = nc.dram_tensor("h_out", [128, NK * B], F32, kind="ExternalOutput")
    c_o = nc.dram_tensor("c_out", [128, NK * B], F32, kind="ExternalOutput")
    attnw_o = nc.dram_tensor("attnw", [B, T, S], F32, kind="ExternalOutput")

    # spill for precomputed lstm_in @ Wx (+bias), bf16, partition-major,
    # gate-permuted (i,g,f,o) so each z-bank reads one contiguous block
    xmm_d = nc.dram_tensor("xmm_spill", [128, T, NM, B], BF16)

    with tile.TileContext(nc) as tc, ExitStack() as ctx:
        persist = ctx.enter_context(tc.tile_pool(name="persist", bufs=1))
        ident_bf = persist.tile([128, 128], BF16)
        masks.make_identity(nc, ident_bf[:])
        ident_f32 = persist.tile([128, 128], F32)
        masks.make_identity(nc, ident_f32[:])

        # hsT: [du(4x128), tokens] bf16 — logits lhsT + h state history
        hsT = persist.tile([128, NK, TOK], BF16)
        cT = persist.tile([128, NK * B], F32)
        hT = persist.tile([128, NK * B], BF16)
        h_last = persist.tile([128, NK * B], F32)

        # big weights: DMA'd once, right at kernel start (no staging, no
        # converts — host supplies bf16)
        wfc_sb = persist.tile([128, NK, VSH], BF16)
        wh_sb = persist.tile([128, NK, G4], BF16)
        for k in range(NK):
            nc.scalar.dma_start(
                wfc_sb[:, k, :], wfc[k * 128:(k + 1) * 128, :]
            )
        nc.scalar.dma_start(wh_sb[:], wh.rearrange("(k p) g -> p k g", p=128))

        # ================= phase A: attention + xmm ======================
        with ExitStack() as pa:
            prep = pa.enter_context(tc.tile_pool(name="prep", bufs=2))
            prep1 = pa.enter_context(tc.tile_pool(name="prep1", bufs=1))

            # ---- small weights + state loads ----
            wq_bf = prep1.tile([128, NE, DU], BF16)
            nc.sync.dma_start(wq_bf[:], wq.rearrange("(k p) d -> p k d", p=128))
            bq_sb = prep1.tile([128, NK], F32)
            nc.sync.dma_start(bq_sb[:], bq.rearrange("(k p) one -> p (k one)", p=128))
            bl_sb = prep1.tile([128, NM], F32)
            nc.sync.dma_start(bl_sb[:], bl.rearrange("(m p) one -> p (m one)", p=128))
            h0_sb = prep.tile([B, DU], F32, tag="h0sb")
            c0_sb = prep.tile([B, DU], F32, tag="c0sb")
            nc.sync.dma_start(h0_sb[:], h0[:, :])
            nc.sync.dma_start(c0_sb[:], c0[:, :])

            # ---- encoder: [s, b, d] bf16 + transposed copy (DMA transpose)
            enc_bf = prep1.tile([S, B, DU], BF16)
            encT_bf = prep1.tile([128, NK, B * S], BF16)
            nc.sync.dma_start(enc_bf[:], enc.rearrange("(b s) d -> s b d", s=S))
            for k in range(NK):
                nc.sync.dma_start(
                    encT_bf[:, k, :], enc[:, k * 128:(k + 1) * 128],
                    transpose=True,
                )

            # ---- token indices (t-major) + embedding gather + transpose ----
            xT = prep1.tile([128, NE, TOK], BF16)
            idx_src = inp.rearrange("b t -> t b")
            with ExitStack() as pg:
                gps = pg.enter_context(
                    tc.tile_pool(name="gps", bufs=2, space="PSUM")
                )
                # h0/c0 -> transposed [128, k*B] via PE
                for k in range(NK):
                    tp = gps.tile([128, 128], F32, tag="tp")
                    nc.tensor.transpose(
                        tp[:, :B], h0_sb[:, k * 128:(k + 1) * 128], ident_f32[:B, :B]
                    )
                    nc.vector.tensor_copy(hT[:, k * B:(k + 1) * B], tp[:, :B])
                    tp2 = gps.tile([128, 128], F32, tag="tp")
                    nc.tensor.transpose(
                        tp2[:, :B], c0_sb[:, k * 128:(k + 1) * 128], ident_f32[:B, :B]
                    )
                    nc.vector.tensor_copy(cT[:, k * B:(k + 1) * B], tp2[:, :B])

                for g in range(NT):
                    idx_sb = prep.tile([128, 1], I32, tag="idx")
                    nc.sync.dma_start(
                        idx_sb[:, :1], idx_src[g * 4:(g + 1) * 4, :]
                    )
                    x_g = prep.tile([128, EMB], BF16, tag="xg")
                    nc.gpsimd.indirect_dma_start(
                        out=x_g[:],
                        out_offset=None,
                        in_=emb[:, :],
                        in_offset=IndirectOffsetOnAxis(ap=idx_sb[:, :1], axis=0),
                    )
                    for e in range(NE):
                        nc.sync.dma_start(
                            xT[:, e, g * 128:(g + 1) * 128],
                            x_g[:, e * 128:(e + 1) * 128],
                            transpose=True,
                        )

                # ---- q GEMM: qT[du, tok] bf16 = Wq.T @ xT (+bq) ----
                qT = prep1.tile([128, NK, TOK], BF16)
                for k in range(NK):
                    for n4 in range(TOK // 512):
                        qp = gps.tile([128, 512], F32, tag="qp")
                        for e in range(NE):
                            nc.tensor.matmul(
                                qp[:],
                                wq_bf[:, e, k * 128:(k + 1) * 128],
                                xT[:, e, n4 * 512:(n4 + 1) * 512],
                                start=(e == 0),
                                stop=(e == NE - 1),
                            )
                        nc.scalar.activation(
                            qT[:, k, n4 * 512:(n4 + 1) * 512],
                            qp[:],
                            AF.Identity,
                            bias=bq_sb[:, k:k + 1],
                        )

            # ---- attention per batch ----
            attT = prep1.tile([128, NK, TOK], BF16)
            with ExitStack() as pat:
                att_ps = pat.enter_context(
                    tc.tile_pool(name="att_ps", bufs=2, space="PSUM")
                )
                att_sb = pat.enter_context(tc.tile_pool(name="att_sb", bufs=4))
                qT_bt = qT.rearrange("p k (t b) -> p k b t", b=B)
                attT_bt = attT.rearrange("p k (t b) -> p k b t", b=B)
                for b in range(B):
                    sc = att_ps.tile([S, 512], F32, tag="sc")
                    for k in range(NK):
                        nc.tensor.matmul(
                            sc[:, :S],
                            qT_bt[:, k, b, :],
                            encT_bf[:, k, b * S:(b + 1) * S],
                            start=(k == 0),
                            stop=(k == NK - 1),
                        )
                    nmax = att_sb.tile([S, 1], F32, tag="nmax")
                    nc.vector.reduce_max(nmax[:], sc[:, :S], axis=AX.X, negate=True)
                    ex = att_sb.tile([S, S], F32, tag="ex")
                    ssum = att_sb.tile([S, 1], F32, tag="ssum")
                    nc.scalar.activation(
                        ex[:], sc[:, :S], AF.Exp,
                        bias=nmax[:, :1], accum_out=ssum[:, :1],
                    )
                    rsum = att_sb.tile([S, 1], F32, tag="rsum")
                    nc.vector.reciprocal(rsum[:], ssum[:])
                    aw = att_sb.tile([S, S], F32, tag="aw")
                    nc.vector.tensor_scalar_mul(aw[:], ex[:], rsum[:, :1])
                    nc.sync.dma_start(attnw_o[b], aw[:])
                    aw_bf = att_sb.tile([S, S], BF16, tag="awbf")
                    nc.vector.tensor_copy(aw_bf[:], aw[:])
                    awt_ps = att_ps.tile([S, 512], BF16, tag="awt")
                    nc.tensor.transpose(
                        awt_ps[:, :S], aw_bf[:], ident_bf[:S, :S]
                    )
                    awt_bf = att_sb.tile([S, S], BF16, tag="awtbf")
                    nc.vector.tensor_copy(awt_bf[:], awt_ps[:, :S])
                    ap_ = att_ps.tile([128, 512], F32, tag="ap")
                    for k in range(NK):
                        nc.tensor.matmul(
                            ap_[:, k * S:(k + 1) * S],
                            enc_bf[:, b, k * 128:(k + 1) * 128],
                            awt_bf[:],
                            start=True,
                            stop=True,
                        )
                    for k in range(NK):
                        nc.vector.tensor_copy(
                            attT_bt[:, k, b, :], ap_[:, k * S:(k + 1) * S]
                        )

            # ---- Wx (bf16, direct) ----
            wx_bf = prep1.tile([128, NX, G4], BF16)
            nc.sync.dma_start(wx_bf[:], wx.rearrange("(k p) g -> p k g", p=128))

            # ---- xmm GEMM -> DRAM spill (lhsT reused across 4 n4 chunks) --
            with ExitStack() as px:
                xm_ps = px.enter_context(
                    tc.tile_pool(name="xm_ps", bufs=2, space="PSUM")
                )
                xm_sb = px.enter_context(tc.tile_pool(name="xm_sb", bufs=3))
                for m in range(NM):
                    xps = [xm_ps.tile([128, 512], F32, tag=f"xp{n4}",
                                      name=f"xp{n4}")
                           for n4 in range(4)]
                    for k in range(NX):
                        for n4 in range(4):
                            lin_k = (
                                attT[:, k, n4 * 512:(n4 + 1) * 512]
                                if k < NK
                                else xT[:, k - NK, n4 * 512:(n4 + 1) * 512]
                            )
                            nc.tensor.matmul(
                                xps[n4][:],
                                wx_bf[:, k, m * 128:(m + 1) * 128],
                                lin_k,
                                start=(k == 0),
                                stop=(k == NX - 1),
                            )
                    m_store = {0: 0, 1: 2, 2: 1, 3: 3}[m // 4] * 4 + m % 4
                    for n4 in range(4):
                        xs = xm_sb.tile([128, 16, B], BF16, tag="xs")
                        nc.scalar.activation(
                            xs[:],
                            xps[n4].rearrange("p (t b) -> p t b", b=B)[:],
                            AF.Identity,
                            bias=bl_sb[:, m:m + 1],
                        )
                        nc.sync.dma_start(
                            xmm_d[:, n4 * 16:(n4 + 1) * 16, m_store, :], xs[:]
                        )

        # ================= phase B: recurrence + logits ==================
        with ExitStack() as pb:
            pb1 = pb.enter_context(tc.tile_pool(name="pb1", bufs=1))
            rring = pb.enter_context(tc.tile_pool(name="rring", bufs=8))
            zps = pb.enter_context(tc.tile_pool(name="zps", bufs=2, space="PSUM"))
            lps = pb.enter_context(tc.tile_pool(name="lps", bufs=2, space="PSUM"))
            gact = pb.enter_context(tc.tile_pool(name="gact", bufs=2))
            lstage = pb.enter_context(tc.tile_pool(name="lstage", bufs=3))

            # bfc replicated across partitions via PE outer product
            bfc_st = pb1.tile([1, VSH], F32)
            nc.sync.dma_start(bfc_st[:], bfc[:, :])
            bfc_bf = pb1.tile([1, VSH], BF16)
            nc.vector.tensor_copy(bfc_bf[:], bfc_st[:])
            ones_bf = pb1.tile([1, 128], BF16)
            nc.vector.memset(ones_bf[:], 1.0)
            bfc128 = pb1.tile([128, VSH], F32)
            for v in range(NV):
                bp = lps.tile([128, VCH], F32, tag="lp0")
                nc.tensor.matmul(
                    bp[:], ones_bf[:], bfc_bf[:, v * VCH:(v + 1) * VCH],
                    start=True, stop=True,
                )
                nc.vector.tensor_copy(bfc128[:, v * VCH:(v + 1) * VCH], bp[:])

            GOFF = {"i": 0, "f": 1, "g": 2, "o": 3}  # gate slice in Wh/Wx

            def emit_logits_pair(gt, vp):
                lp = [lps.tile([128, VCH], F32, tag=f"lp{h}", name=f"lp{h}")
                      for h in range(2)]
                for k in range(NK):
                    for h in range(2):
                        v = vp * 2 + h
                        nc.tensor.matmul(
                            lp[h][:],
                            hsT[:, k, gt * 128:(gt + 1) * 128],
                            wfc_sb[:, k, v * VCH:(v + 1) * VCH],
                            start=(k == 0),
                            stop=(k == NK - 1),
                        )
                for h in range(2):
                    v = vp * 2 + h
                    ls = lstage.tile([128, VCH], F32, tag="ls")
                    nc.vector.scalar_tensor_tensor(
                        out=ls[:],
                        in0=lp[h][:],
                        scalar=0.0,
                        in1=bfc128[:, v * VCH:(v + 1) * VCH],
                        op0=ALU.add,
                        op1=ALU.add,
                    )
                    nc.sync.dma_start(
                        logits_o[gt * 128:(gt + 1) * 128, v * VCH:(v + 1) * VCH],
                        ls[:],
                    )

            hsT_t = hsT.rearrange("p k (t b) -> p k t b", b=B)
            jobs = []
            for t in range(T):
                ring = rring.tile([128, NM, B], BF16, tag="ring")
                nc.sync.dma_start(ring[:], xmm_d[:, t, :, :])
                acts = {}
                # two packed banks: (i, g) and (f, o); per-bank groups stay
                # contiguous; one start=True id-matmul per bank (start is
                # bank-scoped)
                for bi, (bname, pair) in enumerate(
                        (("ig", ("i", "g")), ("fo", ("f", "o")))):
                    zb = zps.tile([128, 256], F32, tag=f"z{bname}",
                                  name=f"z{bname}")
                    nc.tensor.matmul(
                        zb[:],
                        ident_bf[:],
                        ring[:, bi * 8:(bi + 1) * 8, :].rearrange(
                            "p m b -> p (m b)"),
                        start=True,
                        stop=False,
                        skip_group_check=True,
                    )
                    for half, gname in enumerate(pair):
                        go = GOFF[gname]
                        for j in range(4):
                            m = go * 4 + j
                            for k in range(NK):
                                nc.tensor.matmul(
                                    zb[:, half * 128 + j * B:
                                       half * 128 + (j + 1) * B],
                                    wh_sb[:, k, m * 128:(m + 1) * 128],
                                    hT[:, k * B:(k + 1) * B],
                                    start=False,
                                    stop=(j == 3 and k == NK - 1),
                                    skip_group_check=True,
                                )
                    if bname == "ig":
                        a_i = gact.tile([128, 128], F32, tag="ai")
                        nc.scalar.activation(a_i[:], zb[:, 0:128], AF.Sigmoid)
                        a_g = gact.tile([128, 128], F32, tag="ag")
                        nc.scalar.activation(a_g[:], zb[:, 128:256], AF.Tanh)
                        acts["i"], acts["g"] = a_i[:], a_g[:]
                    else:
                        a_fo = gact.tile([128, 256], F32, tag="afo")
                        nc.scalar.activation(a_fo[:], zb[:], AF.Sigmoid)
                        acts["f"], acts["o"] = a_fo[:, 0:128], a_fo[:, 128:256]
                t1 = gact.tile([128, 128], F32, tag="t1")
                nc.vector.tensor_mul(t1[:], acts["i"], acts["g"])
                m1 = gact.tile([128, 128], F32, tag="m1")
                nc.vector.tensor_mul(m1[:], acts["f"], cT[:])
                nc.vector.tensor_add(cT[:], m1[:], t1[:])
                tc_ = gact.tile([128, 128], F32, tag="tc")
                nc.scalar.activation(tc_[:], cT[:], AF.Tanh)
                nc.vector.tensor_mul(hT[:], acts["o"], tc_[:])
                nc.vector.tensor_copy(
                    hsT_t[:, :, t, :], hT.rearrange("p (k b) -> p k b", b=B)[:]
                )
                if t == T - 1:
                    nc.vector.tensor_mul(h_last[:], acts["o"], tc_[:])
                if interleave_logits:
                    if jobs:
                        emit_logits_pair(*jobs.pop(0))
                    if t % 4 == 3:
                        for vp in range(NV // 2):
                            jobs.append((t // 4, vp))

            for gt, vp in (jobs if interleave_logits else
                           [(g, v) for g in range(NT) for v in range(NV // 2)]):
                emit_logits_pair(gt, vp)

            nc.sync.dma_start(h_o[:, :], h_last[:])
            nc.sync.dma_start(c_o[:, :], cT[:])

    nc.compile()
    return nc


_NC_CACHE = {}


def _get_nc():
    if "nc" not in _NC_CACHE:
        _NC_CACHE["nc"] = build_nc()
    return _NC_CACHE["nc"]


def _bf(a):
    return np.ascontiguousarray(np.asarray(a, np.float32).astype(ml_dtypes.bfloat16))


def make_in_maps(inputs, encoder_outputs, h0, c0, emb, Wq, bq, Wx, Wh,
                 b_lstm, Wfc, bfc):
    base = {
        "inputs": np.ascontiguousarray(np.asarray(inputs, np.int32)),
        "enc": _bf(np.asarray(encoder_outputs, np.float32).reshape(B * S, DU)),
        "h0": np.ascontiguousarray(np.asarray(h0, np.float32)),
        "c0": np.ascontiguousarray(np.asarray(c0, np.float32)),
        "emb": _bf(emb),
        "wq": _bf(Wq),
        "bq": np.ascontiguousarray(np.asarray(bq, np.float32).reshape(DU, 1)),
        "wx": _bf(Wx),
        "wh": _bf(Wh),
        "bl": np.ascontiguousarray(np.asarray(b_lstm, np.float32).reshape(G4, 1)),
    }
    Wfc = np.asarray(Wfc, np.float32)
    bfc = np.asarray(bfc, np.float32).reshape(1, V)
    in_maps = []
    for j in range(NCORES):
        m = dict(base)
        m["wfc"] = _bf(Wfc[:, j * VSH:(j + 1) * VSH])
        m["bfc"] = np.ascontiguousarray(bfc[:, j * VSH:(j + 1) * VSH])
        in_maps.append(m)
    return in_maps


def assemble(results):
    logits = np.concatenate(
        [
            results[j]["logits"].reshape(T, B, VSH).transpose(1, 0, 2)
            for j in range(NCORES)
        ],
        axis=-1,
    )
    h = results[0]["h_out"].reshape(128, NK, B).transpose(2, 1, 0).reshape(B, DU)
    c = results[0]["c_out"].reshape(128, NK, B).transpose(2, 1, 0).reshape(B, DU)
    attnw = results[0]["attnw"]
    return logits, (h, c), attnw


def kernel(inputs, encoder_outputs, h0, c0, emb, Wq, bq, Wx, Wh, b_lstm, Wfc, bfc):
    nc = _get_nc()
    in_maps = make_in_maps(inputs, encoder_outputs, h0, c0, emb, Wq, bq, Wx,
                           Wh, b_lstm, Wfc, bfc)
    res = run_bass_kernel_spmd(nc, in_maps, list(range(NCORES)))
    return assemble(res.results)


# revision 25
# speedup vs baseline: 1.0980x; 1.0980x over previous
"""Trainium2 Bass kernel for nn_Decoder (embedding + Luong attention + LSTM + FC).

Sharding (8 NeuronCores):
  - Attention + LSTM replicated on every core (the recurrence is latency-bound,
    not throughput-bound, so replication costs no wall-clock and avoids
    per-step collectives).
  - The dominant cost, logits = hs @ Wfc ([2048,512] @ [512,32000], 262MB fp32
    output), is tensor-parallel over the vocab axis: core j computes columns
    [j*4000, (j+1)*4000).

Layout: token-major, t-major ordering (token = t*B + b) so each LSTM timestep
is a contiguous 32-column slice.  The LSTM recurrence runs in transposed
orientation zT[2048, 32] = Wx-part + Wh.T @ hT with Wh stationary (bf16) and
the per-step xmm slice injected into PSUM via an identity matmul.  The xmm =
lstm_in @ Wx product is precomputed for all timesteps (gate-permuted so each
PSUM bank reads one contiguous block) and spilled to DRAM, streamed back one
timestep per step.  Logits matmuls are interleaved into the recurrence's gate
latency gaps, two vocab chunks per step.  Weights arrive pre-cast to bf16
from the host; fp32 is kept for the cell state, biases and all outputs.
"""

import sys

if "/opt/trn_rl_repo" not in sys.path:
    sys.path.insert(0, "/opt/trn_rl_repo")

from contextlib import ExitStack

import ml_dtypes
import numpy as np

import concourse.bass as bass
import concourse.tile as tile
from concourse import bacc, masks, mybir
from concourse.bass import IndirectOffsetOnAxis
from concourse.bass_utils import run_bass_kernel_spmd

F32 = mybir.dt.float32
BF16 = mybir.dt.bfloat16
I32 = mybir.dt.int32
AF = mybir.ActivationFunctionType
ALU = mybir.AluOpType
AX = mybir.AxisListType

V, EMB, DU = 32000, 256, 512
B, T, S = 32, 64, 64          # batch, Tout, Tin
TOK = B * T                   # 2048 tokens
NT = TOK // 128               # 16 token tiles
NCORES = 8
VSH = V // NCORES             # 4000 vocab cols per core
G4 = 4 * DU                   # 2048 gate dims
NM = G4 // 128                # 16 gate-dim chunks
NK = DU // 128                # 4 contraction chunks over DU
NE = EMB // 128               # 2 contraction chunks over EMB
NX = (DU + EMB) // 128        # 6 contraction chunks over DU+EMB
VCH = 500                     # vocab chunk per psum bank
NV = VSH // VCH               # 8 vocab chunks


def build_nc(interleave_logits=True):
    nc = bacc.Bacc()

    inp = nc.dram_tensor("inputs", [B, T], I32, kind="ExternalInput")
    enc = nc.dram_tensor("enc", [B * S, DU], BF16, kind="ExternalInput")
    h0 = nc.dram_tensor("h0", [B, DU], F32, kind="ExternalInput")
    c0 = nc.dram_tensor("c0", [B, DU], F32, kind="ExternalInput")
    emb = nc.dram_tensor("emb", [V, EMB], BF16, kind="ExternalInput")
    wq = nc.dram_tensor("wq", [EMB, DU], BF16, kind="ExternalInput")
    bq = nc.dram_tensor("bq", [DU, 1], F32, kind="ExternalInput")
    wx = nc.dram_tensor("wx", [DU + EMB, G4], BF16, kind="ExternalInput")
    wh = nc.dram_tensor("wh", [DU, G4], BF16, kind="ExternalInput")
    bl = nc.dram_tensor("bl", [G4, 1], F32, kind="ExternalInput")
    wfc = nc.dram_tensor("wfc", [DU, VSH], BF16, kind="ExternalInput")
    bfc = nc.dram_tensor("bfc", [1, VSH], F32, kind="ExternalInput")

    logits_o = nc.dram_tensor("logits", [TOK, VSH], F32, kind="ExternalOutput")
    h_o = nc.dram_tensor("h_out", [128, NK * B], F32, kind="ExternalOutput")
    c_o = nc.dram_tensor("c_out", [128, NK * B], F32, kind="ExternalOutput")
    attnw_o = nc.dram_tensor("attnw", [B, T, S], F32, kind="ExternalOutput")

    # spill for precomputed lstm_in @ Wx (+bias), bf16, partition-major,
    # gate-permuted (i,g,f,o) so each z-bank reads one contiguous block
    xmm_d = nc.dram_tensor("xmm_spill", [128, T, NM, B], BF16)

    with tile.TileContext(nc) as tc, ExitStack() as ctx:
        persist = ctx.enter_context(tc.tile_pool(name="persist", bufs=1))
        ident_bf = persist.tile([128, 128], BF16)
        masks.make_identity(nc, ident_bf[:])
        ident_f32 = persist.tile([128, 128], F32)
        masks.make_identity(nc, ident_f32[:])

        # hsT: [du(4x128), tokens] bf16 — logits lhsT + h state history
        hsT = persist.tile([128, NK, TOK], BF16)
        cT = persist.tile([128, NK * B], F32)
        hT = persist.tile([128, NK * B], BF16)
        h_last = persist.tile([128, NK * B], F32)

        # big weights: DMA'd once, right at kernel start (no staging, no
        # converts — host supplies bf16)
        wfc_sb = persist.tile([128, NK, VSH], BF16)
        wh_sb = persist.tile([128, NK, G4], BF16)
        for k in range(NK):
            nc.scalar.dma_start(
                wfc_sb[:, k, :], wfc[k * 128:(k + 1) * 128, :]
            )
        nc.scalar.dma_start(wh_sb[:], wh.rearrange("(k p) g -> p k g", p=128))

        # ================= phase A: attention + xmm ======================
        with ExitStack() as pa:
            prep = pa.enter_context(tc.tile_pool(name="prep", bufs=2))
            prep1 = pa.enter_context(tc.tile_pool(name="prep1", bufs=1))

            # ---- small weights + state loads ----
            wq_bf = prep1.tile([128, NE, DU], BF16)
            nc.sync.dma_start(wq_bf[:], wq.rearrange("(k p) d -> p k d", p=128))
            bq_sb = prep1.tile([128, NK], F32)
            nc.sync.dma_start(bq_sb[:], bq.rearrange("(k p) one -> p (k one)", p=128))
            bl_sb = prep1.tile([128, NM], F32)
            nc.sync.dma_start(bl_sb[:], bl.rearrange("(m p) one -> p (m one)", p=128))
            h0_sb = prep.tile([B, DU], F32, tag="h0sb")
            c0_sb = prep.tile([B, DU], F32, tag="c0sb")
            nc.sync.dma_start(h0_sb[:], h0[:, :])
            nc.sync.dma_start(c0_sb[:], c0[:, :])

            # ---- encoder: [s, b, d] bf16 + transposed copy (DMA transpose)
            enc_bf = prep1.tile([S, B, DU], BF16)
            encT_bf = prep1.tile([128, NK, B * S], BF16)
            nc.sync.dma_start(enc_bf[:], enc.rearrange("(b s) d -> s b d", s=S))
            for k in range(NK):
                nc.sync.dma_start(
                    encT_bf[:, k, :], enc[:, k * 128:(k + 1) * 128],
                    transpose=True,
                )

            # ---- token indices (t-major) + embedding gather + transpose ----
            xT = prep1.tile([128, NE, TOK], BF16)
            idx_src = inp.rearrange("b t -> t b")
            with ExitStack() as pg:
                gps = pg.enter_context(
                    tc.tile_pool(name="gps", bufs=2, space="PSUM")
                )
                # h0/c0 -> transposed [128, k*B] via PE
                for k in range(NK):
                    tp = gps.tile([128, 128], F32, tag="tp")
                    nc.tensor.transpose(
                        tp[:, :B], h0_sb[:, k * 128:(k + 1) * 128], ident_f32[:B, :B]
                    )
                    nc.vector.tensor_copy(hT[:, k * B:(k + 1) * B], tp[:, :B])
                    tp2 = gps.tile([128, 128], F32, tag="tp")
                    nc.tensor.transpose(
                        tp2[:, :B], c0_sb[:, k * 128:(k + 1) * 128], ident_f32[:B, :B]
                    )
                    nc.vector.tensor_copy(cT[:, k * B:(k + 1) * B], tp2[:, :B])

                idx_all = prep1.tile([128, NT], I32)
                for g in range(NT):
                    nc.sync.dma_start(
                        idx_all[:, g:g + 1], idx_src[g * 4:(g + 1) * 4, :]
                    )
                for g in range(NT):
                    x_g = prep.tile([128, EMB], BF16, tag="xg", bufs=3)
                    nc.gpsimd.indirect_dma_start(
                        out=x_g[:],
                        out_offset=None,
                        in_=emb[:, :],
                        in_offset=IndirectOffsetOnAxis(ap=idx_all[:, g:g + 1], axis=0),
                    )
                    for e in range(NE):
                        tp3 = gps.tile([128, 128], BF16, tag="tp3")
                        nc.tensor.transpose(
                            tp3[:], x_g[:, e * 128:(e + 1) * 128], ident_bf[:]
                        )
                        nc.vector.tensor_copy(
                            xT[:, e, g * 128:(g + 1) * 128], tp3[:]
                        )

                # ---- q GEMM: qT[du, tok] bf16 = Wq.T @ xT (+bq) ----
                qT = prep1.tile([128, NK, TOK], BF16)
                for k in range(NK):
                    for n4 in range(TOK // 512):
                        qp = gps.tile([128, 512], F32, tag="qp")
                        for e in range(NE):
                            nc.tensor.matmul(
                                qp[:],
                                wq_bf[:, e, k * 128:(k + 1) * 128],
                                xT[:, e, n4 * 512:(n4 + 1) * 512],
                                start=(e == 0),
                                stop=(e == NE - 1),
                            )
                        nc.scalar.activation(
                            qT[:, k, n4 * 512:(n4 + 1) * 512],
                            qp[:],
                            AF.Identity,
                            bias=bq_sb[:, k:k + 1],
                        )

            # ---- attention per batch ----
            attT = prep1.tile([128, NK, TOK], BF16)
            with ExitStack() as pat:
                att_ps = pat.enter_context(
                    tc.tile_pool(name="att_ps", bufs=2, space="PSUM")
                )
                att_sb = pat.enter_context(tc.tile_pool(name="att_sb", bufs=4))
                qT_bt = qT.rearrange("p k (t b) -> p k b t", b=B)
                attT_bt = attT.rearrange("p k (t b) -> p k b t", b=B)
                for b in range(B):
                    sc = att_ps.tile([S, 512], F32, tag="sc")
                    for k in range(NK):
                        nc.tensor.matmul(
                            sc[:, :S],
                            qT_bt[:, k, b, :],
                            encT_bf[:, k, b * S:(b + 1) * S],
                            start=(k == 0),
                            stop=(k == NK - 1),
                        )
                    nmax = att_sb.tile([S, 1], F32, tag="nmax")
                    nc.vector.reduce_max(nmax[:], sc[:, :S], axis=AX.X, negate=True)
                    ex = att_sb.tile([S, S], F32, tag="ex")
                    ssum = att_sb.tile([S, 1], F32, tag="ssum")
                    nc.scalar.activation(
                        ex[:], sc[:, :S], AF.Exp,
                        bias=nmax[:, :1], accum_out=ssum[:, :1],
                    )
                    rsum = att_sb.tile([S, 1], F32, tag="rsum")
                    nc.vector.reciprocal(rsum[:], ssum[:])
                    aw = att_sb.tile([S, S], F32, tag="aw")
                    nc.vector.tensor_scalar_mul(aw[:], ex[:], rsum[:, :1])
                    nc.sync.dma_start(attnw_o[b], aw[:])
                    aw_bf = att_sb.tile([S, S], BF16, tag="awbf")
                    nc.vector.tensor_copy(aw_bf[:], aw[:])
                    awt_ps = att_ps.tile([S, 512], BF16, tag="awt")
                    nc.tensor.transpose(
                        awt_ps[:, :S], aw_bf[:], ident_bf[:S, :S]
                    )
                    awt_bf = att_sb.tile([S, S], BF16, tag="awtbf")
                    nc.vector.tensor_copy(awt_bf[:], awt_ps[:, :S])
                    ap_ = att_ps.tile([128, 512], F32, tag="ap")
                    for k in range(NK):
                        nc.tensor.matmul(
                            ap_[:, k * S:(k + 1) * S],
                            enc_bf[:, b, k * 128:(k + 1) * 128],
                            awt_bf[:],
                            start=True,
                            stop=True,
                        )
                    nc.vector.tensor_copy(
                        attT_bt[:, :, b, :],
                        ap_[:, :NK * S].rearrange("p (k s) -> p k s", s=S)[:],
                    )

            # ---- Wx (bf16, direct) ----
            wx_bf = prep1.tile([128, NX, G4], BF16)
            nc.sync.dma_start(wx_bf[:], wx.rearrange("(k p) g -> p k g", p=128))

            # ---- xmm GEMM -> DRAM spill (lhsT reused across 4 n4 chunks) --
            with ExitStack() as px:
                xm_ps = px.enter_context(
                    tc.tile_pool(name="xm_ps", bufs=2, space="PSUM")
                )
                xm_sb = px.enter_context(tc.tile_pool(name="xm_sb", bufs=3))
                for m in range(NM):
                    xps = [xm_ps.tile([128, 512], F32, tag=f"xp{n4}",
                                      name=f"xp{n4}")
                           for n4 in range(4)]
                    for k in range(NX):
                        for n4 in range(4):
                            lin_k = (
                                attT[:, k, n4 * 512:(n4 + 1) * 512]
                                if k < NK
                                else xT[:, k - NK, n4 * 512:(n4 + 1) * 512]
                            )
                            nc.tensor.matmul(
                                xps[n4][:],
                                wx_bf[:, k, m * 128:(m + 1) * 128],
                                lin_k,
                                start=(k == 0),
                                stop=(k == NX - 1),
                            )
                    m_store = {0: 0, 1: 2, 2: 1, 3: 3}[m // 4] * 4 + m % 4
                    for n4 in range(4):
                        xs = xm_sb.tile([128, 16, B], BF16, tag="xs")
                        nc.scalar.activation(
                            xs[:],
                            xps[n4].rearrange("p (t b) -> p t b", b=B)[:],
                            AF.Identity,
                            bias=bl_sb[:, m:m + 1],
                        )
                        nc.sync.dma_start(
                            xmm_d[:, n4 * 16:(n4 + 1) * 16, m_store, :], xs[:]
                        )

        # ================= phase B: recurrence + logits ==================
        with ExitStack() as pb:
            pb1 = pb.enter_context(tc.tile_pool(name="pb1", bufs=1))
            rring = pb.enter_context(tc.tile_pool(name="rring", bufs=8))
            zps = pb.enter_context(tc.tile_pool(name="zps", bufs=2, space="PSUM"))
            lps = pb.enter_context(tc.tile_pool(name="lps", bufs=2, space="PSUM"))
            gact = pb.enter_context(tc.tile_pool(name="gact", bufs=2))
            lstage = pb.enter_context(tc.tile_pool(name="lstage", bufs=3))

            # bfc replicated across partitions via PE outer product
            bfc_st = pb1.tile([1, VSH], F32)
            nc.sync.dma_start(bfc_st[:], bfc[:, :])
            bfc_bf = pb1.tile([1, VSH], BF16)
            nc.vector.tensor_copy(bfc_bf[:], bfc_st[:])
            ones_bf = pb1.tile([1, 128], BF16)
            nc.vector.memset(ones_bf[:], 1.0)
            bfc128 = pb1.tile([128, VSH], F32)
            for v in range(NV):
                bp = lps.tile([128, VCH], F32, tag="lp0")
                nc.tensor.matmul(
                    bp[:], ones_bf[:], bfc_bf[:, v * VCH:(v + 1) * VCH],
                    start=True, stop=True,
                )
                nc.vector.tensor_copy(bfc128[:, v * VCH:(v + 1) * VCH], bp[:])

            GOFF = {"i": 0, "f": 1, "g": 2, "o": 3}  # gate slice in Wh/Wx

            def emit_logits_pair(gt, vp):
                lp = [lps.tile([128, VCH], F32, tag=f"lp{h}", name=f"lp{h}")
                      for h in range(2)]
                for k in range(NK):
                    for h in range(2):
                        v = vp * 2 + h
                        nc.tensor.matmul(
                            lp[h][:],
                            hsT[:, k, gt * 128:(gt + 1) * 128],
                            wfc_sb[:, k, v * VCH:(v + 1) * VCH],
                            start=(k == 0),
                            stop=(k == NK - 1),
                        )
                for h in range(2):
                    v = vp * 2 + h
                    ls = lstage.tile([128, VCH], F32, tag="ls")
                    nc.vector.scalar_tensor_tensor(
                        out=ls[:],
                        in0=lp[h][:],
                        scalar=0.0,
                        in1=bfc128[:, v * VCH:(v + 1) * VCH],
                        op0=ALU.add,
                        op1=ALU.add,
                    )
                    nc.sync.dma_start(
                        logits_o[gt * 128:(gt + 1) * 128, v * VCH:(v + 1) * VCH],
                        ls[:],
                    )

            hsT_t = hsT.rearrange("p k (t b) -> p k t b", b=B)
            jobs = []
            for t in range(T):
                ring = rring.tile([128, NM, B], BF16, tag="ring")
                nc.sync.dma_start(ring[:], xmm_d[:, t, :, :])
                acts = {}
                # two packed banks: (i, g) and (f, o); per-bank groups stay
                # contiguous; one start=True id-matmul per bank (start is
                # bank-scoped)
                for bi, (bname, pair) in enumerate(
                        (("ig", ("i", "g")), ("fo", ("f", "o")))):
                    zb = zps.tile([128, 256], F32, tag=f"z{bname}",
                                  name=f"z{bname}")
                    nc.tensor.matmul(
                        zb[:],
                        ident_bf[:],
                        ring[:, bi * 8:(bi + 1) * 8, :].rearrange(
                            "p m b -> p (m b)"),
                        start=True,
                        stop=False,
                        skip_group_check=True,
                    )
                    for half, gname in enumerate(pair):
                        go = GOFF[gname]
                        for j in range(4):
                            m = go * 4 + j
                            for k in range(NK):
                                nc.tensor.matmul(
                                    zb[:, half * 128 + j * B:
                                       half * 128 + (j + 1) * B],
                                    wh_sb[:, k, m * 128:(m + 1) * 128],
                                    hT[:, k * B:(k + 1) * B],
                                    start=False,
                                    stop=(j == 3 and k == NK - 1),
                                    skip_group_check=True,
                                )
                    if bname == "ig":
                        a_i = gact.tile([128, 128], F32, tag="ai")
                        nc.scalar.activation(a_i[:], zb[:, 0:128], AF.Sigmoid)
                        a_g = gact.tile([128, 128], F32, tag="ag")
                        nc.scalar.activation(a_g[:], zb[:, 128:256], AF.Tanh)
                        acts["i"], acts["g"] = a_i[:], a_g[:]
                    else:
                        a_fo = gact.tile([128, 256], F32, tag="afo")
                        nc.scalar.activation(a_fo[:], zb[:], AF.Sigmoid)
                        acts["f"], acts["o"] = a_fo[:, 0:128], a_fo[:, 128:256]
                t1 = gact.tile([128, 128], F32, tag="t1")
                nc.vector.tensor_mul(t1[:], acts["i"], acts["g"])
                m1 = gact.tile([128, 128], F32, tag="m1")
                nc.vector.tensor_mul(m1[:], acts["f"], cT[:])
                nc.vector.tensor_add(cT[:], m1[:], t1[:])
                tc_ = gact.tile([128, 128], F32, tag="tc")
                nc.scalar.activation(tc_[:], cT[:], AF.Tanh)
                nc.vector.tensor_mul(hT[:], acts["o"], tc_[:])
                nc.vector.tensor_copy(
                    hsT_t[:, :, t, :], hT.rearrange("p (k b) -> p k b", b=B)[:]
                )
                if t == T - 1:
                    nc.vector.tensor_mul(h_last[:], acts["o"], tc_[:])
                if interleave_logits:
                    if jobs:
                        emit_logits_pair(*jobs.pop(0))
                    if t % 4 == 3:
                        for vp in range(NV // 2):
                            jobs.append((t // 4, vp))

            for gt, vp in (jobs if interleave_logits else
                           [(g, v) for g in range(NT) for v in range(NV // 2)]):
                emit_logits_pair(gt, vp)

            nc.sync.dma_start(h_o[:, :], h_last[:])
            nc.sync.dma_start(c_o[:, :], cT[:])

    nc.compile()
    return nc


_NC_CACHE = {}


def _get_nc():
    if "nc" not in _NC_CACHE:
        _NC_CACHE["nc"] = build_nc()
    return _NC_CACHE["nc"]


def _bf(a):
    return np.ascontiguousarray(np.asarray(a, np.float32).astype(ml_dtypes.bfloat16))


def make_in_maps(inputs, encoder_outputs, h0, c0, emb, Wq, bq, Wx, Wh,
                 b_lstm, Wfc, bfc):
    base = {
        "inputs": np.ascontiguousarray(np.asarray(inputs, np.int32)),
        "enc": _bf(np.asarray(encoder_outputs, np.float32).reshape(B * S, DU)),
        "h0": np.ascontiguousarray(np.asarray(h0, np.float32)),
        "c0": np.ascontiguousarray(np.asarray(c0, np.float32)),
        "emb": _bf(emb),
        "wq": _bf(Wq),
        "bq": np.ascontiguousarray(np.asarray(bq, np.float32).reshape(DU, 1)),
        "wx": _bf(Wx),
        "wh": _bf(Wh),
        "bl": np.ascontiguousarray(np.asarray(b_lstm, np.float32).reshape(G4, 1)),
    }
    Wfc = np.asarray(Wfc, np.float32)
    bfc = np.asarray(bfc, np.float32).reshape(1, V)
    in_maps = []
    for j in range(NCORES):
        m = dict(base)
        m["wfc"] = _bf(Wfc[:, j * VSH:(j + 1) * VSH])
        m["bfc"] = np.ascontiguousarray(bfc[:, j * VSH:(j + 1) * VSH])
        in_maps.append(m)
    return in_maps


def assemble(results):
    logits = np.concatenate(
        [
            results[j]["logits"].reshape(T, B, VSH).transpose(1, 0, 2)
            for j in range(NCORES)
        ],
        axis=-1,
    )
    h = results[0]["h_out"].reshape(128, NK, B).transpose(2, 1, 0).reshape(B, DU)
    c = results[0]["c_out"].reshape(128, NK, B).transpose(2, 1, 0).reshape(B, DU)
    attnw = results[0]["attnw"]
    return logits, (h, c), attnw


def kernel(inputs, encoder_outputs, h0, c0, emb, Wq, bq, Wx, Wh, b_lstm, Wfc, bfc):
    nc = _get_nc()
    in_maps = make_in_maps(inputs, encoder_outputs, h0, c0, emb, Wq, bq, Wx,
                           Wh, b_lstm, Wfc, bfc)
    res = run_bass_kernel_spmd(nc, in_maps, list(range(NCORES)))
    return assemble(res.results)
